# revision 25
# baseline (speedup 1.0000x reference)
"""Trainium2 Bass kernel for nn_MultiHeadFast (multi-head attention with
softmax over the QUERY axis).

Math (faithful to the reference):
  qkv = x @ Ws;  per (b,h):  S[q,k] = Q.K^T,  causal mask k<=q,
  P = softmax_over_q(S * T^-0.5),  out = P @ V.

Layout strategy (v3):
  - Host passes x TRANSPOSED and cast to bf16 (xT: [E, B*T]); host
    un-transposes the output (kernel writes out^T).  No x/out transposes
    on chip.
  - qkv^T = Ws_slice^T x^T via PE matmuls (N=512, full 128x128 mode).
  - S computed TRANSPOSED (S^T[k,q]) so the query-axis softmax reduction
    is fused into the exp (accum_out).  The two heads' S matmuls (K=64)
    are emitted pairwise adjacent -> PE row-tiling runs them CONCURRENTLY
    (measured 2x: 218 -> 110 ns/matmul).  Same for PV (M=64, col-tiling).
  - V natural (tokens on partitions) via DMA-xbar transpose (measured to
    do exact per-128-block transposes), zero PE cost.
  - exp on ScalarE from PSUM in <=1024-col chunks (2x2-bank tiles,
    ping-pong); strips retained in SBUF so PV(b) runs during exp(b1).
  - PSUM: tag "mm" 4x1 bank (QKV accums + PV accums via slot reuse) +
    tag "sps" 2x2 banks = exactly 8 banks.

Sharding: tensor-parallel over heads.  Core c owns heads {2c, 2c+1}; no
collectives.  bf16 inputs, fp32 accumulation (~4e-3 L2 error).
"""

import numpy as np
import ml_dtypes
from contextlib import ExitStack

import concourse.bass as bass
import concourse.mybir as mybir
import concourse.tile as tile
from concourse import bacc
from concourse.bass_utils import run_bass_kernel_spmd

B, T, E = 2, 2048, 1024
H, D = 16, 64
NCORES = 8
HPC = H // NCORES            # heads per core = 2
FPC = HPC * D                # feature cols per core per Q/K/V = 128
P = 128
NT = B * T                   # 4096 tokens total
EK = E // P                  # 8 contraction blocks for QKV
KTILES = T // P              # 16 key tiles per batch
DT = mybir.dt.bfloat16
F32 = mybir.dt.float32
SCALE = float(T) ** -0.5
NEG = -1e30
BF = ml_dtypes.bfloat16


def build_kernel():
    nc = bacc.Bacc("TRN2", target_bir_lowering=False, debug=False)
    xt_dram = nc.dram_tensor("xt", (E, NT), DT, kind="ExternalInput")
    w_dram = nc.dram_tensor("wsl", (E, 3 * FPC), DT, kind="ExternalInput")
    # out^T per batch: [FPC, T]; host transposes back.
    out_dram = nc.dram_tensor("outT", (B, FPC, T), F32, kind="ExternalOutput")

    with tile.TileContext(nc) as tc, ExitStack() as ctx:
        const = ctx.enter_context(tc.tile_pool(name="const", bufs=1))
        big = ctx.enter_context(tc.tile_pool(name="big", bufs=1))
        work = ctx.enter_context(tc.tile_pool(name="work", bufs=2))
        strips = ctx.enter_context(tc.tile_pool(name="strips", bufs=1))
        small = ctx.enter_context(tc.tile_pool(name="small", bufs=8))
        outp = ctx.enter_context(tc.tile_pool(name="outp", bufs=2))
        ps = ctx.enter_context(tc.tile_pool(name="ps", bufs=2, space="PSUM"))

        # ---- constants ----
        zeros_bf = const.tile([P, 512], DT, name="zeros_bf")
        nc.gpsimd.memset(zeros_bf[:], 0.0)
        # diagmask[p, f] = 0 if f >= p else NEG   (keys on partitions, q free)
        diagmask = const.tile([P, P], F32, name="diagmask")
        nc.gpsimd.memset(diagmask[:], 0.0)
        nc.gpsimd.affine_select(
            out=diagmask[:],
            in_=diagmask[:],
            compare_op=mybir.AluOpType.is_ge,
            fill=NEG,
            base=0,
            pattern=[[1, P]],
            channel_multiplier=-1,
        )
        # ---- weights + xT loads, one queue, Q cols + first slab first ----
        wsl = big.tile([P, EK, 3 * FPC], DT, name="wsl")
        xT = big.tile([P, EK, NT], DT, name="xT")

        # host pre-swizzles rows to (ei, eo) order so these DMAs read DRAM
        # sequentially (the (eo, ei) order reads 1KB per 8KB stride)
        def load_wsl(m, eng):
            eng.dma_start(
                wsl[:, :, m * FPC : (m + 1) * FPC],
                w_dram[:, m * FPC : (m + 1) * FPC].rearrange(
                    "(ei eo) f -> ei eo f", ei=P
                ),
            )

        def load_slab(s):
            nc.sync.dma_start(
                xT[:, :, 512 * s : 512 * (s + 1)],
                xt_dram[:, 512 * s : 512 * (s + 1)].rearrange(
                    "(ei eo) t -> ei eo t", ei=P
                ),
            )

        load_wsl(0, nc.sync)
        load_slab(3)
        load_wsl(1, nc.sync)
        load_wsl(2, nc.sync)
        for s in [2, 1, 0, 7, 6, 5, 4]:
            load_slab(s)

        # warm the exp table set during the DMA lead-in
        warm = const.tile([P, 1], F32, name="warm")
        nc.scalar.activation(
            warm[:], diagmask[:, 0:1], mybir.ActivationFunctionType.Exp
        )
        # warm the PE (HAM clock gate: ~3.4us of sustained matmuls takes it
        # from 1.2 to 2.4 GHz) during the DMA lead-in with dummy matmuls
        warm_ps = ps.tile([P, 512], F32, tag="sps", bufs=2, name="warm_ps")
        for _ in range(22):
            nc.tensor.matmul(
                warm_ps[:],
                lhsT=zeros_bf[:, 0:P],
                rhs=zeros_bf[:],
                start=True,
                stop=True,
                skip_group_check=True,
            )

        qt = big.tile([P, NT], DT, name="qt")
        kt = big.tile([P, NT], DT, name="kt")
        v_nat = big.tile([P, NT // P, P], DT, name="v_nat")  # [tok%128, tb, vf]

        # retained per-(b,k,hh) state
        strip_of = {}
        rr_of = {}
        vp_of = {}

        def qkv_slab(b, s):
            """Q/K/V^T projection matmuls for 512-token slab s of batch b;
            V natural via DMA-xbar block transpose (sync queue, needed only
            by the much-later PV phase)."""
            tok0 = b * T + 512 * s
            for m in range(3):
                acc_ps = ps.tile([P, 512], F32, tag="mm", bufs=2, name="qkv_ps")
                for e in range(EK):
                    nc.tensor.matmul(
                        acc_ps[:],
                        lhsT=wsl[:, e, m * P : (m + 1) * P],
                        rhs=xT[:, e, tok0 : tok0 + 512],
                        start=(e == 0),
                        stop=(e == EK - 1),
                    )
                if m == 0:
                    nc.vector.tensor_copy(qt[:, tok0 : tok0 + 512], acc_ps[:])
                elif m == 1:
                    nc.vector.tensor_copy(kt[:, tok0 : tok0 + 512], acc_ps[:])
                else:
                    vt_s = work.tile([P, 512], DT, tag="vt", bufs=2, name="vt_s")
                    nc.vector.tensor_copy(vt_s[:], acc_ps[:])
                    tb0 = (b * T // P) + 4 * s
                    nc.sync.dma_start(
                        v_nat[:, tb0 : tb0 + 4, :], vt_s[:], transpose=True
                    )

        def s_exp(b, k):
            """S^T matmul pairs (row-tiled, both heads concurrent) + mask +
            exp for key-tile k.  Strips + scaled-V retained in SBUF."""
            L = T - P * k  # payload cols (q from 128k to T)
            nch = (L + 1535) // 1536
            cls = (L + 511) // 512  # strip size class 1..4
            st, acc_p = {}, {0: [], 1: []}
            for hh in range(HPC):
                st[hh] = strips.tile(
                    [P, 512 * cls], DT, tag=f"st{cls}",
                    bufs=(10 if cls <= 2 else 9), name=f"st{cls}",
                )
            kb = b * T + P * k
            for c in range(nch):
                co = 1536 * c
                cw = min(1536, L - co)
                sps = [
                    ps.tile([P, 1536], F32, tag="sps", bufs=2, name="sps")
                    for _ in range(HPC)
                ]
                # interleave the two heads' 512-slices so row-tiled pairs
                # are adjacent in the PE queue -> concurrent execution
                for so in range(0, cw, 512):
                    w = min(512, cw - so)
                    qs = kb + co + so
                    for hh in range(HPC):
                        nc.tensor.matmul(
                            sps[hh][:, so : so + w],
                            lhsT=kt[hh * D : (hh + 1) * D, kb : kb + P],
                            rhs=qt[hh * D : (hh + 1) * D, qs : qs + w],
                            start=True,
                            stop=True,
                        )
                for hh in range(HPC):
                    if c == 0:
                        nc.vector.tensor_add(
                            sps[hh][:, 0:P], sps[hh][:, 0:P], diagmask[:]
                        )
                    acc = small.tile([P, 1], F32, tag="acc", name="acc")
                    nc.scalar.activation(
                        st[hh][:, co : co + cw],
                        sps[hh][:, 0:cw],
                        mybir.ActivationFunctionType.Exp,
                        scale=SCALE,
                        accum_out=acc[:],
                    )
                    acc_p[hh].append(acc)
            for hh in range(HPC):
                if len(acc_p[hh]) == 1:
                    ssum = acc_p[hh][0]
                else:
                    ssum = small.tile([P, 1], F32, tag="acc", name="ssum")
                    nc.vector.tensor_add(ssum[:], acc_p[hh][0][:], acc_p[hh][1][:])
                rr = small.tile([P, 1], F32, tag="rr", bufs=72, name="rr")
                nc.vector.reciprocal(rr[:], ssum[:])
                strip_of[(b, k, hh)] = st[hh]
                rr_of[(b, k, hh)] = rr

        def get_vp(b, k, hh):
            if (b, k, hh) not in vp_of:
                vp = small.tile([P, D], DT, tag="vp", bufs=40, name="vp")
                nc.vector.tensor_scalar_mul(
                    vp[:],
                    v_nat[:, (b * T // P) + k, hh * D : (hh + 1) * D],
                    rr_of[(b, k, hh)][:],
                )
                vp_of[(b, k, hh)] = vp
            return vp_of[(b, k, hh)]

        def pv_open(b, slabs):
            """Allocate + zero-init a pair of PV accumulator banks."""
            pv_ps = {
                j: ps.tile([P, 512], F32, tag="mm", bufs=2, name=f"pv_{b}_{j}")
                for j in slabs
            }
            for j in slabs:
                nc.tensor.matmul(
                    pv_ps[j][:],
                    lhsT=zeros_bf[:, 0:P],
                    rhs=zeros_bf[:],
                    start=True,
                    stop=False,
                    skip_group_check=True,
                )
            return pv_ps

        def pv_step(b, pv_ps, slabs, k):
            """PV matmuls for key-tile k into the open slabs.  The two
            heads' matmuls (M=64) are adjacent -> col-tiled concurrency."""
            q0 = P * k
            for j in slabs:
                if j < k // 4:
                    continue
                lo = max(512 * j, q0)
                w = 512 * (j + 1) - lo
                jo = lo - 512 * j
                for hh in range(HPC):
                    nc.tensor.matmul(
                        pv_ps[j][hh * D : (hh + 1) * D, jo : jo + w],
                        lhsT=get_vp(b, k, hh)[:],
                        rhs=strip_of[(b, k, hh)][:, lo - q0 : lo - q0 + w],
                        start=False,
                        stop=(k == 0 and hh == HPC - 1),
                        skip_group_check=True,
                    )

        def pv_close(b, pv_ps, slabs):
            for j in slabs:
                osb = outp.tile([P, 512], F32, tag="osb", name="osb")
                nc.vector.tensor_copy(osb[:], pv_ps[j][:])
                nc.gpsimd.dma_start(
                    out=out_dram[b, :, 512 * j : 512 * (j + 1)], in_=osb[:]
                )

        def pv_pass(b, slabs, kmax):
            pv_ps = pv_open(b, slabs)
            for k in range(kmax, -1, -1):
                pv_step(b, pv_ps, slabs, k)
            pv_close(b, pv_ps, slabs)

        def pv_release(b):
            for k in range(KTILES):
                for hh in range(HPC):
                    strip_of.pop((b, k, hh))
                    rr_of.pop((b, k, hh))
                    vp_of.pop((b, k, hh), None)

        # ---- program order (= scheduler priority) ----
        # per-slab QKV interleaved with S/exp so the Scalar engine (exp) is
        # fed early and never starves; PV batches trail via strip deps.
        for b in range(B):
            for s in range(3, -1, -1):
                qkv_slab(b, s)
                for k in range(4 * s + 3, 4 * s - 1, -1):
                    s_exp(b, k)
        pv_pass(0, (3, 2), KTILES - 1)
        pv_pass(0, (1, 0), 7)
        pv_release(0)
        pv_pass(1, (3, 2), KTILES - 1)
        pv_pass(1, (1, 0), 7)
        pv_release(1)

    nc.compile()
    return nc


_NC_CACHE = None


def _swizzle_rows(a):
    """Reorder rows from e = eo*128+ei to r = ei*EK+eo (see load_* DMAs)."""
    return np.ascontiguousarray(
        a.reshape(EK, P, a.shape[1]).transpose(1, 0, 2).reshape(E, a.shape[1])
    )


def _build_inputs(x: np.ndarray, Ws: np.ndarray):
    x2 = x.reshape(NT, E)
    xt = _swizzle_rows(np.ascontiguousarray(x2.T).astype(BF))
    in_maps = []
    for c in range(NCORES):
        cols = np.concatenate(
            [
                Ws[:, c * FPC : (c + 1) * FPC],
                Ws[:, E + c * FPC : E + (c + 1) * FPC],
                Ws[:, 2 * E + c * FPC : 2 * E + (c + 1) * FPC],
            ],
            axis=1,
        ).astype(BF)
        in_maps.append({"xt": xt, "wsl": _swizzle_rows(cols)})
    return in_maps


def _assemble(results):
    out = np.empty((B, T, H * D), np.float32)
    for c in range(NCORES):
        ot = results[c]["outT"]  # (B, FPC, T) f32
        out[:, :, c * FPC : (c + 1) * FPC] = ot.transpose(0, 2, 1)
    return out


def kernel(x: np.ndarray, Ws: np.ndarray) -> np.ndarray:
    global _NC_CACHE
    if _NC_CACHE is None:
        _NC_CACHE = build_kernel()
    nc = _NC_CACHE
    in_maps = _build_inputs(np.asarray(x, np.float32), np.asarray(Ws, np.float32))
    res = run_bass_kernel_spmd(nc, in_maps, core_ids=list(range(NCORES)))
    return _assemble([res.results[c] for c in range(NCORES)])
